# revision 5
# baseline (speedup 1.0000x reference)
"""NeRF volume-rendering compositing kernel for Trainium2 (8 NeuronCores).

Math (per ray, S=64 samples):
    deltas[i] = t[i+1]-t[i]  (i<63),  deltas[63] = 1e9
    x[i]      = sigmas[i] * deltas[i]
    S[i]      = inclusive cumsum of x          (masked tensor_tensor_scan)
    T[i]      = exp(-S[i])                     (= cumprod(exp(-x)) exactly)
    w[i]      = T[i-1] - T[i]   (T[-1] = 1)    (= alpha * transmittance)
    rgb_out_c = min(sum_i w[i]*rgb[i,c], 1)
    depth     = sum_i w[i]*t[i]

The reference's +1e-9 cumprod stabilizer changes results by ~1e-7 relative,
far below fp32 noise, so the exp(-cumsum) formulation matches.

Layout: rays on partitions. Each core gets 80000 rays = 25 tiles of 3200 rays
([128 partitions x 25 rays-per-row x 64 samples] = [128, 1600] f32 tiles).
Partition p of tile i holds 25 consecutive rays -> every DMA is a contiguous
per-partition block (6400B for t/sigmas, 19200B for rgb).

Engine split per tile (balanced so DVE+GPSIMD finish together):
    GPSIMD: delta sub, x mul (+1e9 tail col), depth product w*t
    VectorE: masked scan (cumsum), w shifted-sub, 3 rgb products, 4 reduces, min
    ScalarE: exp
    Sync(HWDGE): all DMA
"""

import sys

for _p in ("/opt/trn_rl_repo",):
    if _p not in sys.path:
        sys.path.insert(0, _p)

import numpy as np

import concourse.bass as bass
import concourse.bacc as bacc
import concourse.mybir as mybir
from concourse.tile import TileContext
from concourse.bass_utils import run_bass_kernel_spmd

H = 800
W = 800
S = 64
N_CORES = 8
RAYS = H * W                      # 640000
RAYS_PER_CORE = RAYS // N_CORES   # 80000
P = 128                           # partitions
G = 25                            # rays per partition row per tile
TILE_RAYS = P * G                 # 3200
N_TILES = RAYS_PER_CORE // TILE_RAYS  # 25
FD = G * S                        # 1600 free-dim elements per partition
INF = 1.0e9

F32 = mybir.dt.float32
ALU = mybir.AluOpType
ACTF = mybir.ActivationFunctionType
AXIS = mybir.AxisListType


def build_bass():
    nc = bacc.Bacc("TRN2")

    t_in = nc.dram_tensor("t_in", [RAYS_PER_CORE, S], F32, kind="ExternalInput")
    sig_in = nc.dram_tensor("sig_in", [RAYS_PER_CORE, S], F32, kind="ExternalInput")
    rgb_in = nc.dram_tensor("rgb_in", [RAYS_PER_CORE, S * 3], F32, kind="ExternalInput")
    mask_in = nc.dram_tensor("mask_in", [P, FD], F32, kind="ExternalInput")
    rgb_out = nc.dram_tensor("rgb_out", [RAYS_PER_CORE, 3], F32, kind="ExternalOutput")
    d_out = nc.dram_tensor("d_out", [RAYS_PER_CORE], F32, kind="ExternalOutput")

    t_ap = t_in.ap()
    sig_ap = sig_in.ap()
    rgb_ap = rgb_in.ap()
    rgb_out_ap = rgb_out.ap()
    d_out_ap = d_out.ap()

    with TileContext(nc) as tc:
        with (
            tc.tile_pool(name="const", bufs=1) as cpool,
            tc.tile_pool(name="io", bufs=3) as iopool,
            tc.tile_pool(name="mid", bufs=2) as midpool,
            tc.tile_pool(name="stage", bufs=3) as stpool,
        ):
            mask_t = cpool.tile([P, FD], F32, tag="mask")
            nc.sync.dma_start(out=mask_t[:], in_=mask_in.ap())

            for i in range(N_TILES):
                r0 = i * TILE_RAYS
                blk = slice(r0, r0 + TILE_RAYS)

                t_t = iopool.tile([P, FD], F32, tag="t")
                sig_t = iopool.tile([P, FD], F32, tag="sig")
                rgb_t = iopool.tile([P, G * S * 3], F32, tag="rgb")

                nc.sync.dma_start(
                    out=t_t[:], in_=t_ap[blk, :].rearrange("(p g) s -> p (g s)", p=P)
                )
                nc.sync.dma_start(
                    out=sig_t[:], in_=sig_ap[blk, :].rearrange("(p g) s -> p (g s)", p=P)
                )
                nc.sync.dma_start(
                    out=rgb_t[:], in_=rgb_ap[blk, :].rearrange("(p g) c -> p (g c)", p=P)
                )

                t3 = t_t[:].rearrange("p (g s) -> p g s", s=S)
                sig3 = sig_t[:].rearrange("p (g s) -> p g s", s=S)
                rgb4 = rgb_t[:].rearrange("p (g s c) -> p g s c", s=S, c=3)

                # x = sigmas * deltas  (tail col: sigma*1e9)
                delta = midpool.tile([P, FD], F32, tag="delta")
                d3 = delta[:].rearrange("p (g s) -> p g s", s=S)
                nc.gpsimd.tensor_sub(d3[:, :, 0:63], t3[:, :, 1:64], t3[:, :, 0:63])
                nc.gpsimd.memset(d3[:, :, 63:64], INF)
                x_t = midpool.tile([P, FD], F32, tag="x")
                nc.vector.tensor_mul(x_t[:], delta[:], sig_t[:])

                # S = segmented inclusive cumsum: state = (mask * state) + x
                s_t = midpool.tile([P, FD], F32, tag="S")
                nc.vector.tensor_tensor_scan(
                    s_t[:], mask_t[:], x_t[:], 0.0, op0=ALU.mult, op1=ALU.add
                )

                # T = exp(-S)
                T_t = midpool.tile([P, FD], F32, tag="T")
                nc.scalar.activation(T_t[:], s_t[:], ACTF.Exp, scale=-1.0)
                T3 = T_t[:].rearrange("p (g s) -> p g s", s=S)

                # w[i] = T[i-1] - T[i];  w[0] = 1 - T[0]
                w_t = midpool.tile([P, FD], F32, tag="w")
                w3 = w_t[:].rearrange("p (g s) -> p g s", s=S)
                nc.vector.tensor_sub(w3[:, :, 1:64], T3[:, :, 0:63], T3[:, :, 1:64])
                nc.vector.tensor_scalar(
                    w3[:, :, 0:1], T3[:, :, 0:1], 1.0, -1.0,
                    op0=ALU.subtract, op1=ALU.mult,
                )

                # products + per-ray reductions
                st_rgb = stpool.tile([P, G * 3], F32, tag="st_rgb")
                st_rgb3 = st_rgb[:].rearrange("p (g c) -> p g c", c=3)
                st_d = stpool.tile([P, G], F32, tag="st_d")

                p_t = midpool.tile([P, FD], F32, tag="pt")
                nc.gpsimd.tensor_mul(p_t[:], w_t[:], t_t[:])
                p3 = p_t[:].rearrange("p (g s) -> p g s", s=S)
                nc.vector.tensor_reduce(st_d[:], p3, axis=AXIS.X, op=ALU.add)

                for c in range(3):
                    pc = midpool.tile([P, FD], F32, tag="pc")
                    pc3 = pc[:].rearrange("p (g s) -> p g s", s=S)
                    prod_engine = nc.vector if c == 0 else nc.gpsimd
                    prod_engine.tensor_mul(pc3, w3, rgb4[:, :, :, c])
                    nc.vector.tensor_reduce(
                        st_rgb3[:, :, c], pc3, axis=AXIS.X, op=ALU.add
                    )

                nc.vector.tensor_scalar_min(st_rgb[:], st_rgb[:], 1.0)

                nc.sync.dma_start(
                    out=rgb_out_ap[blk, :].rearrange("(p g) c -> p (g c)", p=P),
                    in_=st_rgb[:],
                )
                nc.sync.dma_start(
                    out=d_out_ap[blk].rearrange("(p g) -> p g", p=P),
                    in_=st_d[:],
                )
    nc.compile()
    return nc


def make_mask():
    mask = np.ones((P, FD), dtype=np.float32)
    mask[:, ::S] = 0.0
    return mask


def run_spmd(t, sigmas, rgb, trace=False, **kw):
    """t, sigmas: [RAYS, S] f32; rgb: [RAYS, S*3] f32. Returns (rgb_out, depth, results)."""
    nc = build_bass()
    mask = make_mask()
    in_maps = []
    for c in range(N_CORES):
        sl = slice(c * RAYS_PER_CORE, (c + 1) * RAYS_PER_CORE)
        in_maps.append(
            {
                "t_in": np.ascontiguousarray(t[sl]),
                "sig_in": np.ascontiguousarray(sigmas[sl]),
                "rgb_in": np.ascontiguousarray(rgb[sl]),
                "mask_in": mask,
            }
        )
    res = run_bass_kernel_spmd(nc, in_maps, core_ids=list(range(N_CORES)), trace=trace, **kw)
    rgb_full = np.concatenate([r["rgb_out"] for r in res.results], axis=0)
    d_full = np.concatenate([r["d_out"] for r in res.results], axis=0)
    return rgb_full.reshape(H, W, 3), d_full.reshape(H, W), res


def kernel(rgb, sigmas, t):
    rgb = np.ascontiguousarray(np.asarray(rgb, dtype=np.float32)).reshape(RAYS, S * 3)
    sigmas = np.ascontiguousarray(np.asarray(sigmas, dtype=np.float32)).reshape(RAYS, S)
    t = np.ascontiguousarray(np.asarray(t, dtype=np.float32)).reshape(RAYS, S)
    rgb_m, depth_m, _ = run_spmd(t, sigmas, rgb, trace=False)
    return rgb_m, depth_m


if __name__ == "__main__":
    rng = np.random.default_rng(0)
    rgb = rng.random((H, W, S, 3), dtype=np.float32)
    sig = rng.random((H, W, S), dtype=np.float32) * 2.0
    t = np.linspace(2.0, 6.0, S + 1, dtype=np.float32)[:-1][None, None, :] + 0.0625 * rng.random(
        (H, W, S), dtype=np.float32
    )
    out_rgb, out_d = kernel(rgb, sig, t)
    print(out_rgb.shape, out_d.shape, out_rgb.dtype, out_d.dtype)


# revision 7
# speedup vs baseline: 132.2945x; 132.2945x over previous
"""NeRF volume-rendering compositing kernel for Trainium2 (8 NeuronCores).

Math (per ray, S=64 samples):
    deltas[i] = t[i+1]-t[i]  (i<63),  deltas[63] = 1e9
    x[i]      = sigmas[i] * deltas[i]
    S[i]      = inclusive cumsum of x          (masked tensor_tensor_scan)
    T[i]      = exp(-S[i])                     (= cumprod(exp(-x)) exactly)
    w[i]      = T[i-1] - T[i]   (T[-1] = 1)    (= alpha * transmittance)
    rgb_out_c = min(sum_i w[i]*rgb[i,c], 1)
    depth     = sum_i w[i]*t[i]

The reference's +1e-9 cumprod stabilizer changes results by ~1e-7 relative,
far below fp32 noise, so the exp(-cumsum) formulation matches.

Layout: rays on partitions. Each core gets 80000 rays = 25 tiles of 3200 rays
([128 partitions x 25 rays-per-row x 64 samples] = [128, 1600] f32 tiles).
Partition p of tile i holds 25 consecutive rays -> every DMA is a contiguous
per-partition block (6400B for t/sigmas, 19200B for rgb).

Engine split per tile (balanced so DVE+GPSIMD finish together):
    GPSIMD: delta sub, x mul (+1e9 tail col), depth product w*t
    VectorE: masked scan (cumsum), w shifted-sub, 3 rgb products, 4 reduces, min
    ScalarE: exp
    Sync(HWDGE): all DMA
"""

import sys

for _p in ("/opt/trn_rl_repo",):
    if _p not in sys.path:
        sys.path.insert(0, _p)

import numpy as np

import concourse.bass as bass
import concourse.bacc as bacc
import concourse.mybir as mybir
from concourse.tile import TileContext
from concourse.bass_utils import run_bass_kernel_spmd

H = 800
W = 800
S = 64
N_CORES = 8
RAYS = H * W                      # 640000
RAYS_PER_CORE = RAYS // N_CORES   # 80000
P = 128                           # partitions
G = 25                            # rays per partition row per tile
TILE_RAYS = P * G                 # 3200
N_TILES = RAYS_PER_CORE // TILE_RAYS  # 25
FD = G * S                        # 1600 free-dim elements per partition
INF = 1.0e9

F32 = mybir.dt.float32
ALU = mybir.AluOpType
ACTF = mybir.ActivationFunctionType
AXIS = mybir.AxisListType


def build_bass(reps=1):
    nc = bacc.Bacc("TRN2")

    t_in = nc.dram_tensor("t_in", [RAYS_PER_CORE, S], F32, kind="ExternalInput")
    sig_in = nc.dram_tensor("sig_in", [RAYS_PER_CORE, S], F32, kind="ExternalInput")
    rgb_in = nc.dram_tensor("rgb_in", [RAYS_PER_CORE, S * 3], F32, kind="ExternalInput")
    mask_in = nc.dram_tensor("mask_in", [P, FD], F32, kind="ExternalInput")
    rgb_out = nc.dram_tensor("rgb_out", [RAYS_PER_CORE, 3], F32, kind="ExternalOutput")
    d_out = nc.dram_tensor("d_out", [RAYS_PER_CORE], F32, kind="ExternalOutput")

    t_ap = t_in.ap()
    sig_ap = sig_in.ap()
    rgb_ap = rgb_in.ap()
    rgb_out_ap = rgb_out.ap()
    d_out_ap = d_out.ap()

    with TileContext(nc) as tc:
        with (
            tc.tile_pool(name="const", bufs=1) as cpool,
            tc.tile_pool(name="io", bufs=3) as iopool,
            tc.tile_pool(name="mid", bufs=2) as midpool,
            tc.tile_pool(name="stage", bufs=3) as stpool,
        ):
            mask_t = cpool.tile([P, FD], F32, tag="mask")
            nc.sync.dma_start(out=mask_t[:], in_=mask_in.ap())

            for i in [i % N_TILES for i in range(N_TILES * reps)]:
                r0 = i * TILE_RAYS
                blk = slice(r0, r0 + TILE_RAYS)

                t_t = iopool.tile([P, FD], F32, tag="t")
                sig_t = iopool.tile([P, FD], F32, tag="sig")
                rgb_t = iopool.tile([P, G * S * 3], F32, tag="rgb")

                nc.sync.dma_start(
                    out=t_t[:], in_=t_ap[blk, :].rearrange("(p g) s -> p (g s)", p=P)
                )
                nc.sync.dma_start(
                    out=sig_t[:], in_=sig_ap[blk, :].rearrange("(p g) s -> p (g s)", p=P)
                )
                nc.sync.dma_start(
                    out=rgb_t[:], in_=rgb_ap[blk, :].rearrange("(p g) c -> p (g c)", p=P)
                )

                t3 = t_t[:].rearrange("p (g s) -> p g s", s=S)
                sig3 = sig_t[:].rearrange("p (g s) -> p g s", s=S)
                rgb4 = rgb_t[:].rearrange("p (g s c) -> p g s c", s=S, c=3)

                # x = sigmas * deltas  (tail col: sigma*1e9)
                delta = midpool.tile([P, FD], F32, tag="delta")
                d3 = delta[:].rearrange("p (g s) -> p g s", s=S)
                nc.gpsimd.tensor_sub(d3[:, :, 0:63], t3[:, :, 1:64], t3[:, :, 0:63])
                nc.gpsimd.memset(d3[:, :, 63:64], INF)
                x_t = midpool.tile([P, FD], F32, tag="x")
                nc.vector.tensor_mul(x_t[:], delta[:], sig_t[:])

                # S = segmented inclusive cumsum: state = (mask * state) + x
                s_t = midpool.tile([P, FD], F32, tag="S")
                nc.vector.tensor_tensor_scan(
                    s_t[:], mask_t[:], x_t[:], 0.0, op0=ALU.mult, op1=ALU.add
                )

                # T = exp(-S)
                T_t = midpool.tile([P, FD], F32, tag="T")
                nc.scalar.activation(T_t[:], s_t[:], ACTF.Exp, scale=-1.0)
                T3 = T_t[:].rearrange("p (g s) -> p g s", s=S)

                # w[i] = T[i-1] - T[i];  w[0] = 1 - T[0]
                w_t = midpool.tile([P, FD], F32, tag="w")
                w3 = w_t[:].rearrange("p (g s) -> p g s", s=S)
                nc.vector.tensor_sub(w3[:, :, 1:64], T3[:, :, 0:63], T3[:, :, 1:64])
                nc.vector.tensor_scalar(
                    w3[:, :, 0:1], T3[:, :, 0:1], 1.0, -1.0,
                    op0=ALU.subtract, op1=ALU.mult,
                )

                # products + per-ray reductions
                st_rgb = stpool.tile([P, G * 3], F32, tag="st_rgb")
                st_rgb3 = st_rgb[:].rearrange("p (g c) -> p g c", c=3)
                st_d = stpool.tile([P, G], F32, tag="st_d")

                p_t = midpool.tile([P, FD], F32, tag="pt")
                nc.gpsimd.tensor_mul(p_t[:], w_t[:], t_t[:])
                p3 = p_t[:].rearrange("p (g s) -> p g s", s=S)
                nc.vector.tensor_reduce(st_d[:], p3, axis=AXIS.X, op=ALU.add)

                for c in range(3):
                    pc = midpool.tile([P, FD], F32, tag="pc")
                    pc3 = pc[:].rearrange("p (g s) -> p g s", s=S)
                    prod_engine = nc.vector if c == 0 else nc.gpsimd
                    prod_engine.tensor_mul(pc3, w3, rgb4[:, :, :, c])
                    nc.vector.tensor_reduce(
                        st_rgb3[:, :, c], pc3, axis=AXIS.X, op=ALU.add
                    )

                nc.vector.tensor_scalar_min(st_rgb[:], st_rgb[:], 1.0)

                nc.sync.dma_start(
                    out=rgb_out_ap[blk, :].rearrange("(p g) c -> p (g c)", p=P),
                    in_=st_rgb[:],
                )
                nc.sync.dma_start(
                    out=d_out_ap[blk].rearrange("(p g) -> p g", p=P),
                    in_=st_d[:],
                )
    nc.compile()
    return nc


def make_mask():
    mask = np.ones((P, FD), dtype=np.float32)
    mask[:, ::S] = 0.0
    return mask


def run_spmd(t, sigmas, rgb, trace=False, **kw):
    """t, sigmas: [RAYS, S] f32; rgb: [RAYS, S*3] f32. Returns (rgb_out, depth, results)."""
    nc = build_bass()
    mask = make_mask()
    in_maps = []
    for c in range(N_CORES):
        sl = slice(c * RAYS_PER_CORE, (c + 1) * RAYS_PER_CORE)
        in_maps.append(
            {
                "t_in": np.ascontiguousarray(t[sl]),
                "sig_in": np.ascontiguousarray(sigmas[sl]),
                "rgb_in": np.ascontiguousarray(rgb[sl]),
                "mask_in": mask,
            }
        )
    res = run_bass_kernel_spmd(nc, in_maps, core_ids=list(range(N_CORES)), trace=trace, **kw)
    rgb_full = np.concatenate([r["rgb_out"] for r in res.results], axis=0)
    d_full = np.concatenate([r["d_out"] for r in res.results], axis=0)
    return rgb_full.reshape(H, W, 3), d_full.reshape(H, W), res


def kernel(rgb, sigmas, t):
    rgb = np.ascontiguousarray(np.asarray(rgb, dtype=np.float32)).reshape(RAYS, S * 3)
    sigmas = np.ascontiguousarray(np.asarray(sigmas, dtype=np.float32)).reshape(RAYS, S)
    t = np.ascontiguousarray(np.asarray(t, dtype=np.float32)).reshape(RAYS, S)
    rgb_m, depth_m, _ = run_spmd(t, sigmas, rgb, trace=False)
    return rgb_m, depth_m


if __name__ == "__main__":
    rng = np.random.default_rng(0)
    rgb = rng.random((H, W, S, 3), dtype=np.float32)
    sig = rng.random((H, W, S), dtype=np.float32) * 2.0
    t = np.linspace(2.0, 6.0, S + 1, dtype=np.float32)[:-1][None, None, :] + 0.0625 * rng.random(
        (H, W, S), dtype=np.float32
    )
    out_rgb, out_d = kernel(rgb, sig, t)
    print(out_rgb.shape, out_d.shape, out_rgb.dtype, out_d.dtype)
